# revision 16
# baseline (speedup 1.0000x reference)
"""Trainium2 Bass kernel for nn_DavidBeansV2 (sparse wormhole attention).

Math (per batch item b, derived from the reference):
  xp = x[b, 1:, :]                                  # [P, D]
  q  = l2norm(xp @ Wq + bq); k = l2norm(xp @ Wk + bk)
  S  = q @ k.T + pos_bias    (diag forced very negative)
  topk16 per row of S/TEMP -> softmax weights w (zero elsewhere)
  v  = xp @ Wv + bv
  out[b] = (w / rowsum(w)) @ v                      # [P, D]
The multihead gather+combine with routes shared across heads is exactly a
row-sparse [P,P] x [P,D] matmul, so we compute it densely on the PE with a
masked-softmax weight matrix.

Sharding: data-parallel over batch B=8 across the 8 NeuronCores.

Precision: top-16 selection needs ~1e-6-accurate scores (the 16/17
boundary gaps concentrate near zero), which rules out any single-pass
matmul.  Each exact matmul A@B runs as
    r12(A) @ r12(B)            one fp32r pass   (1 cyc/row, 12-bit operands)
  + [Al' | A'] @ [B' | Bl']    one fp8e5 DoubleRow pass (0.5 cyc/row)
where Al = A - r12(A) and the DoubleRow pair computes Al@B + A@Bl with
power-of-two scale splits so both fp8 products land at natural scale and
accumulate into the same PSUM group.  Normalization is factored out of q/k
and applied to the scores (S = (qraw.kraw) * rq[p] * rk[col] + pb) in fp32
vector ops.  V projection is a single fp32r pass; the combine runs in f16.

Schedule (v2): the PE engine is the bottleneck (~136us of work), so every
other engine is kept off PE's critical path:
  - long PE warmup keeps the pstate clock ramped through the DMA-bound start
  - rk broadcast via Pool partition_broadcast (not PE matmuls)
  - pos_bias fully prefetched + pre-scaled by |q| on Act during V-proj
  - per-block: DVE does s*rk and top-16 only; Pool does +pbq and t=s-sz2;
    Act does exp/wT/out; the PE tail (transpose/combine) is interleaved
    into the next block's score matmuls so Act turnaround never stalls PE.
"""

import numpy as np
import ml_dtypes

import concourse.mybir as mybir
import concourse.tile as tile
from concourse import bass_isa
from concourse import bacc
from concourse.bass_utils import run_bass_kernel_spmd
from concourse.masks import make_identity

F32 = mybir.dt.float32
F32R = mybir.dt.float32r
F16 = mybir.dt.float16
F8E5 = mybir.dt.float8e5
AF = mybir.ActivationFunctionType
OP = mybir.AluOpType
DR = mybir.MatmulPerfMode.DoubleRow
E5 = ml_dtypes.float8_e5m2

B, P, D = 8, 1024, 768
TEMP = 0.1
KC = D // 128     # 6 contraction chunks
PB = P // 128     # 8 row blocks
MINVAL = -50.0    # match_replace fill; below any real score, above diag fill
DIAGVAL = -10000.0
ASC = 2.0 ** 6    # xl cross scale: (xl*ASC) @ (Wr/ASC)
BSC = 2.0 ** 10   # Wl cross scale: (x/BSC) @ (Wl*BSC)
NWARM = 22        # PE warmup matmuls ([1,512] each) covering the input DMA


def build_program(with_bias: bool):
    nc = bacc.Bacc(
        "TRN2",
        target_bir_lowering=False,
        debug=False,
        enable_asserts=False,
        num_devices=B,
    )
    xr_d = nc.dram_tensor("xr", [D, P], F32R, kind="ExternalInput").ap()
    x8_d = nc.dram_tensor("x8", [D, 2, P], F8E5, kind="ExternalInput").ap()
    wqr_d = nc.dram_tensor("wqr", [D, D], F32R, kind="ExternalInput").ap()
    wq8_d = nc.dram_tensor("wq8", [D, 2, D], F8E5, kind="ExternalInput").ap()
    wkr_d = nc.dram_tensor("wkr", [D, D], F32R, kind="ExternalInput").ap()
    wk8_d = nc.dram_tensor("wk8", [D, 2, D], F8E5, kind="ExternalInput").ap()
    wvr_d = nc.dram_tensor("wvr", [D, D], F32R, kind="ExternalInput").ap()
    pb = nc.dram_tensor("pb", [P, P], F32, kind="ExternalInput").ap()
    if with_bias:
        bqkv = nc.dram_tensor("bqkv", [1, 3, D], F32, kind="ExternalInput").ap()
    out = nc.dram_tensor("out", [P, D], F16, kind="ExternalOutput").ap()

    with tile.TileContext(nc) as tc:
        consts = tc.alloc_tile_pool(name="consts", bufs=1)
        persist = tc.alloc_tile_pool(name="persist", bufs=1)
        wq_pool = tc.alloc_tile_pool(name="wq_pool", bufs=1)
        inp_pool = tc.alloc_tile_pool(name="inp", bufs=1)
        work2 = tc.alloc_tile_pool(name="work2", bufs=1)
        wk_pool = tc.alloc_tile_pool(name="wk_pool", bufs=1)
        psum2 = tc.alloc_tile_pool(name="psum2", bufs=1, space="PSUM")

        # ---- PE warmup: wide matmuls on a zeroed operand keep the PE busy
        # (and its pstate clock ramping) through the initial input-DMA wait.
        warm_op = consts.tile([128, 448], F16, tag="warm_op")
        nc.gpsimd.memset(warm_op, 0.0)
        warm_ps = psum2.tile([1, 448], F32, tag="warm_ps")
        for _ in range(NWARM):
            nc.tensor.matmul(warm_ps, warm_op[:, 0:1], warm_op,
                             start=True, stop=True)

        ident = consts.tile([128, 128], F16, tag="ident")
        make_identity(nc, ident)
        ones_row = consts.tile([1, 512 if with_bias else 128], F32,
                               tag="ones_row")
        nc.vector.memset(ones_row, 1.0)

        # ---- load inputs, chunked so the first matmuls start early ----
        xr_sb = inp_pool.tile([128, KC, P], F32R, tag="xr_sb", name="xr_sb")
        x8_sb = inp_pool.tile([128, KC, 2, P], F8E5, tag="x8_sb", name="x8_sb")
        wqr_sb = wq_pool.tile([128, KC, D], F32R, tag="wqr_sb", name="wqr_sb")
        wq8_sb = wq_pool.tile([128, KC, 2, D], F8E5, tag="wq8_sb", name="wq8_sb")
        wkr_sb = wk_pool.tile([128, KC, D], F32R, tag="wkr_sb", name="wkr_sb")
        wk8_sb = wk_pool.tile([128, KC, 2, D], F8E5, tag="wk8_sb", name="wk8_sb")

        xr_src = xr_d.rearrange("(o p) f -> p o f", p=128)
        x8_src = x8_d.rearrange("(o p) t f -> p o t f", p=128)
        wqr_src = wqr_d.rearrange("(o p) f -> p o f", p=128)
        wq8_src = wq8_d.rearrange("(o p) t f -> p o t f", p=128)
        wkr_src = wkr_d.rearrange("(o p) f -> p o f", p=128)
        wk8_src = wk8_d.rearrange("(o p) t f -> p o t f", p=128)

        # phase order is k-proj, q-proj, V, blocks; loads are sequenced so
        # each arriving chunk unlocks the next group of matmuls
        h0, h1 = slice(0, 512), slice(512, P)
        for dc in range(KC):
            nc.sync.dma_start(xr_sb[:, dc, h0], xr_src[:, dc, h0])
            nc.sync.dma_start(wkr_sb[:, dc, :], wkr_src[:, dc, :])
        for dc in range(KC):
            nc.sync.dma_start(x8_sb[:, dc, :, h0], x8_src[:, dc, :, h0])
            nc.sync.dma_start(wk8_sb[:, dc, :, :], wk8_src[:, dc, :, :])
        for dc in range(KC):
            nc.sync.dma_start(xr_sb[:, dc, h1], xr_src[:, dc, h1])
        for dc in range(KC):
            nc.sync.dma_start(x8_sb[:, dc, :, h1], x8_src[:, dc, :, h1])
        for dc in range(KC):
            nc.sync.dma_start(wqr_sb[:, dc, :], wqr_src[:, dc, :])
        for dc in range(KC):
            nc.sync.dma_start(wq8_sb[:, dc, :, :], wq8_src[:, dc, :, :])
        if with_bias:
            bias_sb = consts.tile([1, 3, D], F32, tag="bias_sb")
            nc.sync.dma_start(bias_sb, bqkv)

        # persistent q/k operands for the scores stage
        q_r = persist.tile([128, KC, P], F32R, tag="q_r", name="q_r")
        k_r = persist.tile([128, KC, P], F32R, tag="k_r", name="k_r")
        q_8 = persist.tile([128, KC, 2, P], F8E5, tag="q_8", name="q_8")
        k_8 = persist.tile([128, KC, 2, P], F8E5, tag="k_8", name="k_8")
        v_sb = persist.tile([128, PB, D], F16, tag="v_sb")
        rk_bcast = persist.tile([128, P], F32, tag="rk_bcast")
        wpack = {"q": (wqr_sb, wq8_sb), "k": (wkr_sb, wk8_sb)}
        rpack = {"q": (q_r, q_8, 0, 1), "k": (k_r, k_8, 1, 0)}
        bidx = {"q": 0, "k": 1}
        rinv_rows = {}

        sq_accs = {}

        def emit_proj(nm):
            """Raw projection (f32r + fp8 DoubleRow crosses) + squares.

            The six 128-row output blocks are processed as two ping-pong SETS
            of three PSUM groups: while set B's matmuls run (~5.8us), set A's
            consumers (Act/DVE/Pool splits+squares) drain, so no sweep ever
            stalls on its own consumer chain."""
            ti = bidx[nm]
            wr, w8 = wpack[nm]
            t_r, t_8, l_slot, full_slot = rpack[nm]
            sq_acc = work2.tile([128, P], F32, tag=f"sq_{nm}")
            sq_accs[nm] = sq_acc
            for sl in range(2):
                s = slice(sl * 512, (sl + 1) * 512)
                for half in range(2):
                    dbs = list(range(3 * half, 3 * half + 3))
                    mm = {db: psum2.tile([128, 512], F32, tag=f"mmh{db}",
                                         name=f"mmh{db}", bufs=1)
                          for db in dbs}
                    # f32r sweep dc-major: consumes each arriving x/w chunk
                    for dc in range(KC):
                        for db in dbs:
                            nc.tensor.matmul(
                                mm[db],
                                wr[:, dc, db * 128:(db + 1) * 128],
                                xr_sb[:, dc, s],
                                start=(dc == 0),
                                stop=False,
                            )
                    # DR sweep dblk-major: groups close staggered
                    for db in dbs:
                        for dc in range(KC):
                            nc.tensor.matmul(
                                mm[db],
                                w8[:, dc, :, db * 128:(db + 1) * 128],
                                x8_sb[:, dc, :, s],
                                start=False,
                                stop=(dc == KC - 1) and not with_bias,
                                perf_mode=DR,
                            )
                        if with_bias:
                            nc.tensor.matmul(
                                mm[db],
                                bias_sb[:, ti, db * 128:(db + 1) * 128],
                                ones_row,
                                start=False,
                                stop=True,
                            )
                        # split raw projection into f32r hi + fp8 pair; the
                        # sq-mul runs FIRST on DVE (it feeds the norm chain,
                        # which gates rk/rq and ultimately the block phase)
                        nc.scalar.activation(t_r[:, db, s], mm[db], AF.Identity)
                        # norm^2 via q_raw*r12(q_raw): 1.3e-6 rel, row-uniform
                        if db == 0:
                            nc.vector.tensor_mul(sq_acc[:, s], mm[db],
                                                 t_r[:, db, s].bitcast(F32))
                        else:
                            sq_full = work2.tile([128, P], F32, tag="allr",
                                                 name="sq_full")
                            nc.vector.tensor_mul(sq_full[:, 0:512], mm[db],
                                                 t_r[:, db, s].bitcast(F32))
                            nc.gpsimd.tensor_add(sq_acc[:, s], sq_acc[:, s],
                                                 sq_full[:, 0:512])
                        nc.scalar.activation(t_8[:, db, full_slot, s], mm[db],
                                             AF.Identity)
                        nc.vector.tensor_sub(t_8[:, db, l_slot, s], mm[db],
                                             t_r[:, db, s].bitcast(F32))

        nq_rows = {}
        # ln and the Newton scratch share one row (disjoint lifetimes); all
        # rows stay base-0 (vector ops need equal input base partitions)
        rowsA = work2.tile([1, P], F32, tag="rowsA", name="rowsA")
        rinv_k_sb = work2.tile([1, P], F32, tag="rinv_k", name="rinv_k")
        rinv_q_sb = work2.tile([1, P], F32, tag="rinv_q", name="rinv_q")
        nq_sb = work2.tile([1, P], F32, tag="nq_q", name="nq_q")

        def emit_norm_head(nm):
            """Reduce squares -> seed 1/|row| via the ACT Rsqrt spline.
            (Rsqrt's table set also holds identity, so the projection copies
            never force an act-table reload around it.)"""
            sq_acc = sq_accs[nm]
            allr = work2.tile([128, P], F32, tag="allr")
            nc.gpsimd.partition_all_reduce(allr, sq_acc, channels=128,
                                           reduce_op=bass_isa.ReduceOp.add)
            norm2_sb = allr[0:1, :]
            rinv_row = rinv_k_sb if nm == "k" else rinv_q_sb
            nc.vector.reciprocal(rinv_row, norm2_sb)
            nc.scalar.activation(rinv_row, rinv_row, AF.Sqrt)
            rinv_rows[nm] = rinv_row
            return norm2_sb

        def emit_norm_tail(nm, norm2_sb):
            """Two Newton steps (3 fused ops each: spline seeds are too loose
            for the flip-sensitive top-16); for q also |row| itself."""
            rinv_row = rinv_rows[nm]
            rr = rowsA[0:1, :]
            for _ in range(2):
                nc.vector.tensor_mul(rr, rinv_row, rinv_row)
                # rr = (-0.5 * rinv^2) * norm2
                nc.vector.scalar_tensor_tensor(rr, rr, -0.5, norm2_sb,
                                               op0=OP.mult, op1=OP.mult)
                # rinv = (rr + 1.5) * rinv
                nc.vector.scalar_tensor_tensor(rinv_row, rr, 1.5, rinv_row,
                                               op0=OP.add, op1=OP.mult)
            if nm == "q":
                # |q_p| itself: scales pb so the score rows can stay raw
                nc.vector.tensor_mul(nq_sb, norm2_sb, rinv_row)
                nq_rows[nm] = nq_sb

        # ---- k projection (chases the x/wk input stream) ----
        emit_proj("k")
        norm2_k = emit_norm_head("k")

        # Wv loads into the arena wk_pool frees (queued after the wq loads);
        # pos_bias prefetch follows, landing well before the block phase.
        wk_pool.release()
        wv_pool = tc.alloc_tile_pool(name="wv_pool", bufs=1)
        wvr_sb = wv_pool.tile([128, KC, D], F32R, tag="wvr_sb", name="wvr_sb")
        wvr_src = wvr_d.rearrange("(o p) f -> p o f", p=128)
        for dc in range(KC):
            nc.sync.dma_start(wvr_sb[:, dc, :], wvr_src[:, dc, :])

        # ---- q projection (wq loaded during k) ----
        emit_proj("q")
        norm2_q = emit_norm_head("q")
        # k's Newton runs on DVE after ALL q-proj consumers (so it never
        # blocks the in-order DVE queue ahead of PSUM-bank recycling)
        emit_norm_tail("k", norm2_k)
        nc.gpsimd.partition_broadcast(rk_bcast, rinv_rows["k"], channels=128)

        # ---- v projection (single f32r pass, natural [p, d] layout) ----
        for pblk in range(PB):
            vh = [psum2.tile([128, 512], F32, tag=f"mmh{(2 * pblk + i) % KC}",
                             name=f"mmh{(2 * pblk + i) % KC}", bufs=1)
                  for i in range(2)]
            for dc in range(KC):
                for sl, s, n in ((0, slice(0, 512), 512), (1, slice(512, D), 256)):
                    nc.tensor.matmul(
                        vh[sl][:, :n],
                        xr_sb[:, dc, pblk * 128:(pblk + 1) * 128],
                        wvr_sb[:, dc, s],
                        start=(dc == 0),
                        stop=(dc == KC - 1) and not with_bias,
                    )
            if with_bias:
                for sl, s, n in ((0, slice(0, 512), 512), (1, slice(512, D), 256)):
                    nc.tensor.matmul(
                        vh[sl][:, :n],
                        ones_row[:, :128],
                        bias_sb[:, 2, s],
                        start=False,
                        stop=True,
                    )
            nc.scalar.activation(v_sb[:, pblk, 0:512], vh[0], AF.Identity)
            nc.scalar.activation(v_sb[:, pblk, 512:D], vh[1][:, :256], AF.Identity)
            if pblk == 1:
                # q's Newton chain runs on the otherwise-idle DVE here
                emit_norm_tail("q", norm2_q)
        wv_pool.release()

        # ---- rq relayout: [rq | nq] rows -> per-partition columns ----
        # (kept on PE: 16 one-column matmuls; q's Newton finished during V)
        rqx_cols = persist.tile([128, 2, PB], F32, tag="rqx_cols")
        rqT_cols = persist.tile([128, PB], F32, tag="rqT_cols")
        nrqT_cols = persist.tile([128, PB], F32, tag="nrqT_cols")
        rq_ps = psum2.tile([128, 2, PB], F32, tag="rq_ps", name="rq_ps",
                           bufs=1)
        for si, row in ((0, rinv_rows["q"]), (1, nq_rows["q"])):
            for j in range(PB):
                nc.tensor.matmul(
                    rq_ps[:, si, j:j + 1],
                    row[:, j * 128:(j + 1) * 128],
                    ones_row[:, 0:1],
                    start=True,
                    stop=True,
                )
        nc.scalar.activation(rqx_cols, rq_ps, AF.Identity)
        # rq/TEMP and -rq/TEMP columns for the exp scale/bias
        nc.vector.tensor_scalar_mul(rqT_cols, rqx_cols[:, 0, :], 1.0 / TEMP)
        nc.vector.tensor_scalar_mul(nrqT_cols, rqx_cols[:, 0, :], -1.0 / TEMP)

        work2.release()
        inp_pool.release()
        wq_pool.release()
        psum2.release()

        # ---- per row-block: scores, top-16 softmax, combine ----
        work3 = tc.alloc_tile_pool(name="work3", bufs=2)
        wpool = tc.alloc_tile_pool(name="wpool", bufs=3)
        pb_pool = tc.alloc_tile_pool(name="pb_pool", bufs=3)
        psum3 = tc.alloc_tile_pool(name="psum3", bufs=1, space="PSUM")

        state = {}

        def emit_pb_load(pblk):
            """Prefetch one pos_bias row block (3-deep rotation)."""
            pb_t = pb_pool.tile([128, P], F32, tag="pb_sb")
            nc.sync.dma_start(pb_t, pb[pblk * 128:(pblk + 1) * 128, :])
            state.setdefault(pblk, {})["pb"] = pb_t

        def emit_prep_half(pblk, sl):
            """S matmuls for one 512-half + DVE rk-scale + Pool pb-add."""
            pbs = slice(pblk * 128, (pblk + 1) * 128)
            if sl == 0:
                s_ps = psum3.tile([128, P], F32, tag="s_ps", name="s_ps",
                                  bufs=2)
                s_sb = work3.tile([128, P], F32, tag="s_sb")
                state.setdefault(pblk, {}).update(s_ps=s_ps, s_sb=s_sb)
            else:
                s_ps = state[pblk]["s_ps"]
                s_sb = state[pblk]["s_sb"]
            s = slice(sl * 512, (sl + 1) * 512)
            for dc in range(KC):
                nc.tensor.matmul(
                    s_ps[:, s],
                    q_r[:, dc, pbs],
                    k_r[:, dc, s],
                    start=(dc == 0),
                    stop=False,
                )
            for dc in range(KC):
                nc.tensor.matmul(
                    s_ps[:, s],
                    q_8[:, dc, :, pbs],
                    k_8[:, dc, :, s],
                    start=False,
                    stop=(dc == KC - 1),
                    perf_mode=DR,
                )
            # s' = Sraw*rk + pb*|q| ; rows stay scaled by |q_p|.  The |q|
            # scale folds into the Pool add (one scalar_tensor_tensor).
            nc.vector.tensor_mul(s_sb[:, s], s_ps[:, s], rk_bcast[:, s])
            nc.gpsimd.scalar_tensor_tensor(
                s_sb[:, s], state[pblk]["pb"][:, s],
                rqx_cols[:, 1, pblk:pblk + 1], s_sb[:, s],
                op0=OP.mult, op1=OP.add)

        def emit_topk(pblk):
            """top-16 (DVE) + t = s - sz2 (Pool) + exp (Act) for pblk."""
            s_sb = state[pblk]["s_sb"]
            # top-16 per row: two rounds of max8 + match_replace
            m8a = work3.tile([128, 8], F32, tag="m8a")
            nc.vector.max(m8a, s_sb)
            sz1 = work3.tile([128, P], F32, tag="sz1")
            nc.vector.match_replace(sz1, in_to_replace=m8a, in_values=s_sb,
                                    imm_value=MINVAL)
            m8b = work3.tile([128, 8], F32, tag="m8b")
            nc.vector.max(m8b, sz1)
            sz2 = work3.tile([128, P], F32, tag="sz2")
            nc.vector.match_replace(sz2, in_to_replace=m8b, in_values=sz1,
                                    imm_value=MINVAL)
            # w = exp((T + MINVAL - m) * rq/TEMP); off-top entries underflow
            ebias = work3.tile([128, 1], F32, tag="ebias")
            nc.vector.tensor_scalar_add(ebias, m8a[:, 0:1], -MINVAL)
            nc.vector.tensor_mul(ebias, ebias, nrqT_cols[:, pblk:pblk + 1])
            if pblk >= 1:
                emit_recip(pblk - 1)
            # T = s - sz2: 0 off the top-16, s - MINVAL on it (Pool)
            t_sb = work3.tile([128, P], F32, tag="t_sb")
            nc.gpsimd.tensor_sub(t_sb, s_sb, sz2)
            w_sb = wpool.tile([128, P], F16, tag="w_sb", bufs=4)
            den = wpool.tile([128, 1], F32, tag="den", bufs=4)
            nc.scalar.activation(w_sb, t_sb, AF.Exp, bias=ebias,
                                 scale=rqT_cols[:, pblk:pblk + 1],
                                 accum_out=den)
            state[pblk].update(w_sb=w_sb, den=den)

        def emit_recip(pblk):
            rden = wpool.tile([128, 1], F32, tag="rden", bufs=4)
            nc.vector.reciprocal(rden, state[pblk]["den"])
            state[pblk]["rden"] = rden

        def emit_tp(pblk):
            """PE transpose of w + Act copy to SBUF."""
            w_sb = state[pblk]["w_sb"]
            tp_ps = psum3.tile([128, P], F16, tag="tp_ps", name="tp_ps",
                               bufs=2)
            for qc in range(PB):
                nc.tensor.transpose(
                    tp_ps[:, qc * 128:(qc + 1) * 128],
                    w_sb[:, qc * 128:(qc + 1) * 128],
                    ident,
                )
            wT_sb = work3.tile([128, P], F16, tag="wT_sb")
            nc.scalar.activation(wT_sb, tp_ps, AF.Identity)
            state[pblk]["wT"] = wT_sb

        def emit_combine(pblk):
            """PE combine with v, scale by 1/den, store."""
            wT_sb, rden = state[pblk]["wT"], state[pblk]["rden"]
            pbs = slice(pblk * 128, (pblk + 1) * 128)
            o_ps = psum3.tile([128, D], F32, tag="o_ps", name="o_ps", bufs=1)
            for qc in range(PB):
                for sl, s in ((0, slice(0, 512)), (1, slice(512, D))):
                    nc.tensor.matmul(
                        o_ps[:, s],
                        wT_sb[:, qc * 128:(qc + 1) * 128],
                        v_sb[:, qc, s],
                        start=(qc == 0),
                        stop=(qc == PB - 1),
                    )
            out_sb = work3.tile([128, D], F16, tag="out_sb")
            nc.scalar.activation(out_sb, o_ps, AF.Identity, scale=rden)
            nc.sync.dma_start(out[pbs, :], out_sb)

        # software pipeline: per macro-step i the PE runs
        #   prep_i(h0) | tp_{i-3} | prep_i(h1) | combine_{i-3}
        # so the Act wT turnaround for block i-3 overlaps prep_i(h1), and
        # DVE's topk_{i-1} overlaps both prep halves.  The tail lag of 3
        # absorbs the pipeline-fill bubble of the first block's DVE chain.
        emit_pb_load(0)
        emit_pb_load(1)
        emit_pb_load(2)
        for i in range(PB):
            emit_prep_half(i, 0)
            if i >= 3:
                emit_tp(i - 3)
            if i >= 1:
                emit_topk(i - 1)
            emit_prep_half(i, 1)
            if i >= 3:
                emit_combine(i - 3)
            if i + 3 <= PB - 1:
                emit_pb_load(i + 3)
        # drain: tails for blocks 5,6,7 and the last top-k chain
        emit_tp(PB - 3)
        emit_topk(PB - 1)
        emit_combine(PB - 3)
        emit_tp(PB - 2)
        emit_recip(PB - 1)
        emit_combine(PB - 2)
        emit_tp(PB - 1)
        emit_combine(PB - 1)

        pb_pool.release()
        wpool.release()
        work3.release()
        psum3.release()
        persist.release()
        consts.release()

    nc.finalize()
    return nc


_PROG_CACHE = {}


def _r12(a):
    """Round fp32 values to 12 explicit mantissa bits (= f32r rounding)."""
    m, e = np.frexp(np.asarray(a, np.float64))
    return ((np.round(m * 4096.0) / 4096.0) * np.exp2(e)).astype(np.float32)


def _w_pack(W):
    """f32r hi part + fp8e5 DoubleRow pair [(Wr/ASC), (Wl*BSC)] for a weight."""
    W = np.asarray(W, dtype=np.float32)
    Wr = _r12(W)
    Wl = (W.astype(np.float64) - Wr).astype(np.float32)
    pair = np.stack([(Wr / ASC).astype(E5), (Wl * BSC).astype(E5)], axis=1)
    return np.ascontiguousarray(Wr), np.ascontiguousarray(pair)


def kernel(**inputs) -> np.ndarray:
    x = np.ascontiguousarray(np.asarray(inputs["x"], dtype=np.float32))
    Wq = np.asarray(inputs["Wq"], dtype=np.float32)
    Wk = np.asarray(inputs["Wk"], dtype=np.float32)
    Wv = np.asarray(inputs["Wv"], dtype=np.float32)
    bq = np.asarray(inputs["bq"], dtype=np.float32)
    bk = np.asarray(inputs["bk"], dtype=np.float32)
    bv = np.asarray(inputs["bv"], dtype=np.float32)
    pos_bias = np.asarray(inputs["pos_bias"], dtype=np.float32)

    with_bias = bool(np.any(bq) or np.any(bk) or np.any(bv))

    # Diagonal is excluded by the reference (set to -1e9 before top-k); any
    # value below every real score gives the identical top-16 and weights.
    pb_adj = np.ascontiguousarray(pos_bias.copy())
    np.fill_diagonal(pb_adj, DIAGVAL)

    if with_bias not in _PROG_CACHE:
        _PROG_CACHE[with_bias] = build_program(with_bias)
    nc = _PROG_CACHE[with_bias]

    Wqr, Wq8 = _w_pack(Wq)
    Wkr, Wk8 = _w_pack(Wk)
    Wvr = np.ascontiguousarray(_r12(Wv))

    in_maps = []
    for b in range(B):
        xTb = np.ascontiguousarray(x[b, 1:, :].T)
        xr = _r12(xTb)
        xl = (xTb.astype(np.float64) - xr).astype(np.float32)
        x8 = np.ascontiguousarray(
            np.stack([(xl * ASC).astype(E5), (xTb / BSC).astype(E5)], axis=1))
        m = {
            "xr": np.ascontiguousarray(xr), "x8": x8,
            "wqr": Wqr, "wq8": Wq8, "wkr": Wkr, "wk8": Wk8, "wvr": Wvr,
            "pb": pb_adj,
        }
        if with_bias:
            m["bqkv"] = np.ascontiguousarray(np.stack([bq, bk, bv])[None])
        in_maps.append(m)

    res = run_bass_kernel_spmd(nc, in_maps, core_ids=list(range(B)))
    return np.stack([res.results[b]["out"] for b in range(B)]).astype(np.float32)


# revision 19
# speedup vs baseline: 1.0490x; 1.0490x over previous
"""Trainium2 Bass kernel for nn_DavidBeansV2 (sparse wormhole attention).

Math (per batch item b, derived from the reference):
  xp = x[b, 1:, :]                                  # [P, D]
  q  = l2norm(xp @ Wq + bq); k = l2norm(xp @ Wk + bk)
  S  = q @ k.T + pos_bias    (diag forced very negative)
  topk16 per row of S/TEMP -> softmax weights w (zero elsewhere)
  v  = xp @ Wv + bv
  out[b] = (w / rowsum(w)) @ v                      # [P, D]
The multihead gather+combine with routes shared across heads is exactly a
row-sparse [P,P] x [P,D] matmul, so we compute it densely on the PE with a
masked-softmax weight matrix.

Sharding: data-parallel over batch B=8 across the 8 NeuronCores.

Precision: top-16 selection needs ~1e-6-accurate scores (the 16/17
boundary gaps concentrate near zero), which rules out any single-pass
matmul.  Each exact matmul A@B runs as
    r12(A) @ r12(B)            one fp32r pass   (1 cyc/row, 12-bit operands)
  + [Al' | A'] @ [B' | Bl']    one fp8e5 DoubleRow pass (0.5 cyc/row)
where Al = A - r12(A) and the DoubleRow pair computes Al@B + A@Bl with
power-of-two scale splits so both fp8 products land at natural scale and
accumulate into the same PSUM group.  Normalization is factored out of q/k
and applied to the scores (S = (qraw.kraw) * rq[p] * rk[col] + pb) in fp32
vector ops.  V projection is a single fp32r pass; the combine runs in f16.

Schedule (v2): the PE engine is the bottleneck (~136us of work), so every
other engine is kept off PE's critical path:
  - long PE warmup keeps the pstate clock ramped through the DMA-bound start
  - rk broadcast via Pool partition_broadcast (not PE matmuls)
  - pos_bias fully prefetched + pre-scaled by |q| on Act during V-proj
  - per-block: DVE does s*rk and top-16 only; Pool does +pbq and t=s-sz2;
    Act does exp/wT/out; the PE tail (transpose/combine) is interleaved
    into the next block's score matmuls so Act turnaround never stalls PE.
"""

import numpy as np
import ml_dtypes

import concourse.mybir as mybir
import concourse.tile as tile
from concourse import bass_isa
from concourse import bacc
from concourse.bass_utils import run_bass_kernel_spmd
from concourse.masks import make_identity

F32 = mybir.dt.float32
F32R = mybir.dt.float32r
F16 = mybir.dt.float16
F8E5 = mybir.dt.float8e5
AF = mybir.ActivationFunctionType
OP = mybir.AluOpType
DR = mybir.MatmulPerfMode.DoubleRow
E5 = ml_dtypes.float8_e5m2

B, P, D = 8, 1024, 768
TEMP = 0.1
KC = D // 128     # 6 contraction chunks
PB = P // 128     # 8 row blocks
MINVAL = -50.0    # match_replace fill; below any real score, above diag fill
DIAGVAL = -10000.0
ASC = 2.0 ** 6    # xl cross scale: (xl*ASC) @ (Wr/ASC)
BSC = 2.0 ** 10   # Wl cross scale: (x/BSC) @ (Wl*BSC)
NWARM = 22        # PE warmup matmuls ([1,512] each) covering the input DMA


def build_program(with_bias: bool):
    nc = bacc.Bacc(
        "TRN2",
        target_bir_lowering=False,
        debug=False,
        enable_asserts=False,
        num_devices=B,
    )
    xr_d = nc.dram_tensor("xr", [D, P], F32R, kind="ExternalInput").ap()
    x8_d = nc.dram_tensor("x8", [D, 2, P], F8E5, kind="ExternalInput").ap()
    wqr_d = nc.dram_tensor("wqr", [D, D], F32R, kind="ExternalInput").ap()
    wq8_d = nc.dram_tensor("wq8", [D, 2, D], F8E5, kind="ExternalInput").ap()
    wkr_d = nc.dram_tensor("wkr", [D, D], F32R, kind="ExternalInput").ap()
    wk8_d = nc.dram_tensor("wk8", [D, 2, D], F8E5, kind="ExternalInput").ap()
    wvr_d = nc.dram_tensor("wvr", [D, D], F32R, kind="ExternalInput").ap()
    pb = nc.dram_tensor("pb", [P, P], F32, kind="ExternalInput").ap()
    if with_bias:
        bqkv = nc.dram_tensor("bqkv", [1, 3, D], F32, kind="ExternalInput").ap()
    out = nc.dram_tensor("out", [P, D], F16, kind="ExternalOutput").ap()

    with tile.TileContext(nc) as tc:
        consts = tc.alloc_tile_pool(name="consts", bufs=1)
        persist = tc.alloc_tile_pool(name="persist", bufs=1)
        wq_pool = tc.alloc_tile_pool(name="wq_pool", bufs=1)
        inp_pool = tc.alloc_tile_pool(name="inp", bufs=1)
        work2 = tc.alloc_tile_pool(name="work2", bufs=1)
        wk_pool = tc.alloc_tile_pool(name="wk_pool", bufs=1)
        psum2 = tc.alloc_tile_pool(name="psum2", bufs=1, space="PSUM")

        # ---- PE warmup: wide matmuls on a zeroed operand keep the PE busy
        # (and its pstate clock ramping) through the initial input-DMA wait.
        warm_op = consts.tile([128, 448], F16, tag="warm_op")
        nc.gpsimd.memset(warm_op, 0.0)
        warm_ps = psum2.tile([1, 448], F32, tag="warm_ps")
        for _ in range(NWARM):
            nc.tensor.matmul(warm_ps, warm_op[:, 0:1], warm_op,
                             start=True, stop=True)

        ident = consts.tile([128, 128], F16, tag="ident")
        make_identity(nc, ident)
        ones_row = consts.tile([1, 512 if with_bias else 128], F32,
                               tag="ones_row")
        nc.vector.memset(ones_row, 1.0)

        # ---- load inputs, chunked so the first matmuls start early ----
        xr_sb = inp_pool.tile([128, KC, P], F32R, tag="xr_sb", name="xr_sb")
        x8_sb = inp_pool.tile([128, KC, 2, P], F8E5, tag="x8_sb", name="x8_sb")
        wqr_sb = wq_pool.tile([128, KC, D], F32R, tag="wqr_sb", name="wqr_sb")
        wq8_sb = wq_pool.tile([128, KC, 2, D], F8E5, tag="wq8_sb", name="wq8_sb")
        wkr_sb = wk_pool.tile([128, KC, D], F32R, tag="wkr_sb", name="wkr_sb")
        wk8_sb = wk_pool.tile([128, KC, 2, D], F8E5, tag="wk8_sb", name="wk8_sb")

        xr_src = xr_d.rearrange("(o p) f -> p o f", p=128)
        x8_src = x8_d.rearrange("(o p) t f -> p o t f", p=128)
        wqr_src = wqr_d.rearrange("(o p) f -> p o f", p=128)
        wq8_src = wq8_d.rearrange("(o p) t f -> p o t f", p=128)
        wkr_src = wkr_d.rearrange("(o p) f -> p o f", p=128)
        wk8_src = wk8_d.rearrange("(o p) t f -> p o t f", p=128)

        # phase order is k-proj, q-proj, V, blocks; loads are sequenced so
        # each arriving chunk unlocks the next group of matmuls
        h0, h1 = slice(0, 512), slice(512, P)
        for dc in range(KC):
            nc.sync.dma_start(xr_sb[:, dc, h0], xr_src[:, dc, h0])
            nc.sync.dma_start(wkr_sb[:, dc, :], wkr_src[:, dc, :])
        for dc in range(KC):
            nc.sync.dma_start(x8_sb[:, dc, :, h0], x8_src[:, dc, :, h0])
            nc.sync.dma_start(wk8_sb[:, dc, :, :], wk8_src[:, dc, :, :])
        for dc in range(KC):
            nc.sync.dma_start(xr_sb[:, dc, h1], xr_src[:, dc, h1])
        for dc in range(KC):
            nc.sync.dma_start(x8_sb[:, dc, :, h1], x8_src[:, dc, :, h1])
        for dc in range(KC):
            nc.sync.dma_start(wqr_sb[:, dc, :], wqr_src[:, dc, :])
        for dc in range(KC):
            nc.sync.dma_start(wq8_sb[:, dc, :, :], wq8_src[:, dc, :, :])
        if with_bias:
            bias_sb = consts.tile([1, 3, D], F32, tag="bias_sb")
            nc.sync.dma_start(bias_sb, bqkv)

        # persistent q/k operands for the scores stage
        q_r = persist.tile([128, KC, P], F32R, tag="q_r", name="q_r")
        k_r = persist.tile([128, KC, P], F32R, tag="k_r", name="k_r")
        q_8 = persist.tile([128, KC, 2, P], F8E5, tag="q_8", name="q_8")
        k_8 = persist.tile([128, KC, 2, P], F8E5, tag="k_8", name="k_8")
        v_sb = persist.tile([128, PB, D], F16, tag="v_sb")
        rk_bcast = persist.tile([128, P], F32, tag="rk_bcast")
        wpack = {"q": (wqr_sb, wq8_sb), "k": (wkr_sb, wk8_sb)}
        rpack = {"q": (q_r, q_8, 0, 1), "k": (k_r, k_8, 1, 0)}
        bidx = {"q": 0, "k": 1}
        rinv_rows = {}

        sq_accs = {}

        def emit_proj(nm):
            """Raw projection (f32r + fp8 DoubleRow crosses) + squares.

            The six 128-row output blocks are processed as two ping-pong SETS
            of three PSUM groups: while set B's matmuls run (~5.8us), set A's
            consumers (Act/DVE/Pool splits+squares) drain, so no sweep ever
            stalls on its own consumer chain."""
            ti = bidx[nm]
            wr, w8 = wpack[nm]
            t_r, t_8, l_slot, full_slot = rpack[nm]
            sq_acc = work2.tile([128, P], F32, tag=f"sq_{nm}")
            sq_accs[nm] = sq_acc
            for sl in range(2):
                s = slice(sl * 512, (sl + 1) * 512)
                for half in range(2):
                    dbs = list(range(3 * half, 3 * half + 3))
                    mm = {db: psum2.tile([128, 512], F32, tag=f"mmh{db}",
                                         name=f"mmh{db}", bufs=1)
                          for db in dbs}
                    # f32r sweep dc-major: consumes each arriving x/w chunk
                    for dc in range(KC):
                        for db in dbs:
                            nc.tensor.matmul(
                                mm[db],
                                wr[:, dc, db * 128:(db + 1) * 128],
                                xr_sb[:, dc, s],
                                start=(dc == 0),
                                stop=False,
                            )
                    # DR sweep dblk-major: groups close staggered
                    for db in dbs:
                        for dc in range(KC):
                            nc.tensor.matmul(
                                mm[db],
                                w8[:, dc, :, db * 128:(db + 1) * 128],
                                x8_sb[:, dc, :, s],
                                start=False,
                                stop=(dc == KC - 1) and not with_bias,
                                perf_mode=DR,
                            )
                        if with_bias:
                            nc.tensor.matmul(
                                mm[db],
                                bias_sb[:, ti, db * 128:(db + 1) * 128],
                                ones_row,
                                start=False,
                                stop=True,
                            )
                        # split raw projection into f32r hi + fp8 pair; the
                        # sq-mul runs FIRST on DVE (it feeds the norm chain,
                        # which gates rk/rq and ultimately the block phase)
                        nc.scalar.activation(t_r[:, db, s], mm[db], AF.Identity)
                        # norm^2 via q_raw*r12(q_raw): 1.3e-6 rel, row-uniform
                        if db == 0:
                            nc.vector.tensor_mul(sq_acc[:, s], mm[db],
                                                 t_r[:, db, s].bitcast(F32))
                        else:
                            sq_full = work2.tile([128, P], F32, tag="allr",
                                                 name="sq_full")
                            nc.vector.tensor_mul(sq_full[:, 0:512], mm[db],
                                                 t_r[:, db, s].bitcast(F32))
                            nc.gpsimd.tensor_add(sq_acc[:, s], sq_acc[:, s],
                                                 sq_full[:, 0:512])
                        nc.scalar.activation(t_8[:, db, full_slot, s], mm[db],
                                             AF.Identity)
                        nc.vector.tensor_sub(t_8[:, db, l_slot, s], mm[db],
                                             t_r[:, db, s].bitcast(F32))

        nq_rows = {}
        # ln and the Newton scratch share one row (disjoint lifetimes); all
        # rows stay base-0 (vector ops need equal input base partitions)
        rowsA = work2.tile([1, P], F32, tag="rowsA", name="rowsA")
        rinv_k_sb = work2.tile([1, P], F32, tag="rinv_k", name="rinv_k")
        rinv_q_sb = work2.tile([1, P], F32, tag="rinv_q", name="rinv_q")
        nq_sb = None  # allocated lazily in q's Newton tail (reuses rinv_k arena)

        def emit_norm_reduce(nm):
            """Partition-reduce the squares on Pool (early, off DVE).
            q's reduce target reuses sq_k's arena (dead after k's reduce)."""
            sq_acc = sq_accs[nm]
            allr = work2.tile([128, P], F32,
                              tag="allr_k" if nm == "k" else "sq_k")
            nc.gpsimd.partition_all_reduce(allr, sq_acc, channels=128,
                                           reduce_op=bass_isa.ReduceOp.add)
            return allr[0:1, :]

        def emit_norm_seed(nm, norm2_sb):
            """Seed 1/|row| = sqrt(1/norm2).  (sqrt's table set also holds
            identity, so projection copies never force an act reload.)"""
            rinv_row = rinv_k_sb if nm == "k" else rinv_q_sb
            nc.vector.reciprocal(rinv_row, norm2_sb)
            nc.scalar.activation(rinv_row, rinv_row, AF.Sqrt)
            rinv_rows[nm] = rinv_row

        def emit_norm_tail(nm, norm2_sb):
            """Two Newton steps (3 fused ops each: spline seeds are too loose
            for the flip-sensitive top-16); for q also |row| itself."""
            rinv_row = rinv_rows[nm]
            rr = rowsA[0:1, :]
            for _ in range(2):
                nc.vector.tensor_mul(rr, rinv_row, rinv_row)
                # rr = (-0.5 * rinv^2) * norm2
                nc.vector.scalar_tensor_tensor(rr, rr, -0.5, norm2_sb,
                                               op0=OP.mult, op1=OP.mult)
                # rinv = (rr + 1.5) * rinv
                nc.vector.scalar_tensor_tensor(rinv_row, rr, 1.5, rinv_row,
                                               op0=OP.add, op1=OP.mult)
            if nm == "q":
                # |q_p| itself: scales pb so the score rows can stay raw
                # (arena of rinv_k, which is dead after k's broadcast)
                nq_row = work2.tile([1, P], F32, tag="rinv_k", name="nq_q")
                nc.vector.tensor_mul(nq_row, norm2_sb, rinv_row)
                nq_rows[nm] = nq_row

        # ---- k projection (chases the x/wk input stream) ----
        emit_proj("k")
        norm2_k = emit_norm_reduce("k")

        # Wv loads into the arena wk_pool frees (queued after the wq loads);
        # pos_bias prefetch follows, landing well before the block phase.
        wk_pool.release()
        wv_pool = tc.alloc_tile_pool(name="wv_pool", bufs=1)
        wvr_sb = wv_pool.tile([128, KC, D], F32R, tag="wvr_sb", name="wvr_sb")
        wvr_src = wvr_d.rearrange("(o p) f -> p o f", p=128)
        for dc in range(KC):
            nc.sync.dma_start(wvr_sb[:, dc, :], wvr_src[:, dc, :])

        # ---- q projection (wq loaded during k) ----
        emit_proj("q")
        norm2_q = emit_norm_reduce("q")
        # k's rsqrt seed + Newton run on DVE after ALL q-proj consumers (so
        # they never block the in-order DVE queue ahead of PSUM recycling)
        emit_norm_seed("k", norm2_k)
        emit_norm_tail("k", norm2_k)
        nc.gpsimd.partition_broadcast(rk_bcast, rinv_rows["k"], channels=128)

        # ---- v projection (single f32r pass, natural [p, d] layout) ----
        for pblk in range(PB):
            vh = [psum2.tile([128, 512], F32, tag=f"mmh{(2 * pblk + i) % KC}",
                             name=f"mmh{(2 * pblk + i) % KC}", bufs=1)
                  for i in range(2)]
            for dc in range(KC):
                for sl, s, n in ((0, slice(0, 512), 512), (1, slice(512, D), 256)):
                    nc.tensor.matmul(
                        vh[sl][:, :n],
                        xr_sb[:, dc, pblk * 128:(pblk + 1) * 128],
                        wvr_sb[:, dc, s],
                        start=(dc == 0),
                        stop=(dc == KC - 1) and not with_bias,
                    )
            if with_bias:
                for sl, s, n in ((0, slice(0, 512), 512), (1, slice(512, D), 256)):
                    nc.tensor.matmul(
                        vh[sl][:, :n],
                        ones_row[:, :128],
                        bias_sb[:, 2, s],
                        start=False,
                        stop=True,
                    )
            nc.scalar.activation(v_sb[:, pblk, 0:512], vh[0], AF.Identity)
            nc.scalar.activation(v_sb[:, pblk, 512:D], vh[1][:, :256], AF.Identity)
            if pblk == 1:
                # q's seed + Newton chain runs on the otherwise-idle DVE here
                emit_norm_seed("q", norm2_q)
                emit_norm_tail("q", norm2_q)
        wv_pool.release()

        # ---- rq relayout: [rq | nq] rows -> per-partition columns ----
        # (kept on PE: 16 one-column matmuls; q's Newton finished during V)
        rqx_cols = persist.tile([128, 2, PB], F32, tag="rqx_cols")
        rqT_cols = persist.tile([128, PB], F32, tag="rqT_cols")
        nrqT_cols = persist.tile([128, PB], F32, tag="nrqT_cols")
        rq_ps = psum2.tile([128, 2, PB], F32, tag="rq_ps", name="rq_ps",
                           bufs=1)
        for si, row in ((0, rinv_rows["q"]), (1, nq_rows["q"])):
            for j in range(PB):
                nc.tensor.matmul(
                    rq_ps[:, si, j:j + 1],
                    row[:, j * 128:(j + 1) * 128],
                    ones_row[:, 0:1],
                    start=True,
                    stop=True,
                )
        nc.scalar.activation(rqx_cols, rq_ps, AF.Identity)
        # rq/TEMP and -rq/TEMP columns for the exp scale/bias
        nc.vector.tensor_scalar_mul(rqT_cols, rqx_cols[:, 0, :], 1.0 / TEMP)
        nc.vector.tensor_scalar_mul(nrqT_cols, rqx_cols[:, 0, :], -1.0 / TEMP)

        work2.release()
        inp_pool.release()
        wq_pool.release()
        psum2.release()

        # ---- per row-block: scores, top-16 softmax, combine ----
        work3 = tc.alloc_tile_pool(name="work3", bufs=2)
        wpool = tc.alloc_tile_pool(name="wpool", bufs=3)
        pb_pool = tc.alloc_tile_pool(name="pb_pool", bufs=3)
        psum3 = tc.alloc_tile_pool(name="psum3", bufs=1, space="PSUM")

        state = {}

        def emit_pb_load(pblk):
            """Prefetch one pos_bias row block (3-deep rotation)."""
            pb_t = pb_pool.tile([128, P], F32, tag="pb_sb")
            nc.sync.dma_start(pb_t, pb[pblk * 128:(pblk + 1) * 128, :])
            state.setdefault(pblk, {})["pb"] = pb_t

        def emit_prep_half(pblk, sl):
            """S matmuls for one 512-half + DVE rk-scale + Pool pb-add."""
            pbs = slice(pblk * 128, (pblk + 1) * 128)
            if sl == 0:
                s_ps = psum3.tile([128, P], F32, tag="s_ps", name="s_ps",
                                  bufs=2)
                s_sb = work3.tile([128, P], F32, tag="s_sb")
                state.setdefault(pblk, {}).update(s_ps=s_ps, s_sb=s_sb)
            else:
                s_ps = state[pblk]["s_ps"]
                s_sb = state[pblk]["s_sb"]
            s = slice(sl * 512, (sl + 1) * 512)
            for dc in range(KC):
                nc.tensor.matmul(
                    s_ps[:, s],
                    q_r[:, dc, pbs],
                    k_r[:, dc, s],
                    start=(dc == 0),
                    stop=False,
                )
            for dc in range(KC):
                nc.tensor.matmul(
                    s_ps[:, s],
                    q_8[:, dc, :, pbs],
                    k_8[:, dc, :, s],
                    start=False,
                    stop=(dc == KC - 1),
                    perf_mode=DR,
                )
            # s' = Sraw*rk + pb*|q| ; rows stay scaled by |q_p|.  The |q|
            # scale folds into the Pool add (one scalar_tensor_tensor).
            nc.vector.tensor_mul(s_sb[:, s], s_ps[:, s], rk_bcast[:, s])
            nc.gpsimd.scalar_tensor_tensor(
                s_sb[:, s], state[pblk]["pb"][:, s],
                rqx_cols[:, 1, pblk:pblk + 1], s_sb[:, s],
                op0=OP.mult, op1=OP.add)

        def emit_topk(pblk):
            """top-16 (DVE) + t = s - sz2 (Pool) + exp (Act) for pblk."""
            s_sb = state[pblk]["s_sb"]
            # top-16 per row: two rounds of max8 + match_replace
            m8a = work3.tile([128, 8], F32, tag="m8a")
            nc.vector.max(m8a, s_sb)
            sz1 = work3.tile([128, P], F32, tag="sz1")
            nc.vector.match_replace(sz1, in_to_replace=m8a, in_values=s_sb,
                                    imm_value=MINVAL)
            m8b = work3.tile([128, 8], F32, tag="m8b")
            nc.vector.max(m8b, sz1)
            sz2 = work3.tile([128, P], F32, tag="sz2")
            nc.vector.match_replace(sz2, in_to_replace=m8b, in_values=sz1,
                                    imm_value=MINVAL)
            # w = exp((T + MINVAL - m) * rq/TEMP); off-top entries underflow
            ebias = work3.tile([128, 1], F32, tag="ebias")
            nc.vector.tensor_scalar_add(ebias, m8a[:, 0:1], -MINVAL)
            nc.vector.tensor_mul(ebias, ebias, nrqT_cols[:, pblk:pblk + 1])
            if pblk >= 1:
                emit_recip(pblk - 1)
            # T = s - sz2: 0 off the top-16, s - MINVAL on it (Pool)
            t_sb = work3.tile([128, P], F32, tag="t_sb")
            nc.gpsimd.tensor_sub(t_sb, s_sb, sz2)
            w_sb = wpool.tile([128, P], F16, tag="w_sb", bufs=4)
            den = wpool.tile([128, 1], F32, tag="den", bufs=4)
            nc.scalar.activation(w_sb, t_sb, AF.Exp, bias=ebias,
                                 scale=rqT_cols[:, pblk:pblk + 1],
                                 accum_out=den)
            state[pblk].update(w_sb=w_sb, den=den)

        def emit_recip(pblk):
            rden = wpool.tile([128, 1], F32, tag="rden", bufs=4)
            nc.vector.reciprocal(rden, state[pblk]["den"])
            state[pblk]["rden"] = rden

        def emit_tp(pblk):
            """PE transpose of w + Act copy to SBUF."""
            w_sb = state[pblk]["w_sb"]
            tp_ps = psum3.tile([128, P], F16, tag="tp_ps", name="tp_ps",
                               bufs=2)
            for qc in range(PB):
                nc.tensor.transpose(
                    tp_ps[:, qc * 128:(qc + 1) * 128],
                    w_sb[:, qc * 128:(qc + 1) * 128],
                    ident,
                )
            wT_sb = work3.tile([128, P], F16, tag="wT_sb")
            nc.scalar.activation(wT_sb, tp_ps, AF.Identity)
            state[pblk]["wT"] = wT_sb

        def emit_combine(pblk):
            """PE combine with v, scale by 1/den, store."""
            wT_sb, rden = state[pblk]["wT"], state[pblk]["rden"]
            pbs = slice(pblk * 128, (pblk + 1) * 128)
            o_ps = psum3.tile([128, D], F32, tag="o_ps", name="o_ps", bufs=1)
            for qc in range(PB):
                for sl, s in ((0, slice(0, 512)), (1, slice(512, D))):
                    nc.tensor.matmul(
                        o_ps[:, s],
                        wT_sb[:, qc * 128:(qc + 1) * 128],
                        v_sb[:, qc, s],
                        start=(qc == 0),
                        stop=(qc == PB - 1),
                    )
            out_sb = work3.tile([128, D], F16, tag="out_sb")
            nc.scalar.activation(out_sb, o_ps, AF.Identity, scale=rden)
            nc.sync.dma_start(out[pbs, :], out_sb)

        # software pipeline: per macro-step i the PE runs
        #   prep_i(h0) | tp_{i-3} | prep_i(h1) | combine_{i-3}
        # so the Act wT turnaround for block i-3 overlaps prep_i(h1), and
        # DVE's topk_{i-1} overlaps both prep halves.  The tail lag of 3
        # absorbs the pipeline-fill bubble of the first block's DVE chain.
        emit_pb_load(0)
        emit_pb_load(1)
        emit_pb_load(2)
        for i in range(PB):
            emit_prep_half(i, 0)
            if i >= 3:
                emit_tp(i - 3)
            if i >= 1:
                emit_topk(i - 1)
            emit_prep_half(i, 1)
            if i >= 3:
                emit_combine(i - 3)
            if i + 3 <= PB - 1:
                emit_pb_load(i + 3)
        # drain: tails for blocks 5,6,7 and the last top-k chain
        emit_tp(PB - 3)
        emit_topk(PB - 1)
        emit_combine(PB - 3)
        emit_tp(PB - 2)
        emit_recip(PB - 1)
        emit_combine(PB - 2)
        emit_tp(PB - 1)
        emit_combine(PB - 1)

        pb_pool.release()
        wpool.release()
        work3.release()
        psum3.release()
        persist.release()
        consts.release()

    nc.finalize()
    return nc


_PROG_CACHE = {}


def _r12(a):
    """Round fp32 values to 12 explicit mantissa bits (= f32r rounding)."""
    m, e = np.frexp(np.asarray(a, np.float64))
    return ((np.round(m * 4096.0) / 4096.0) * np.exp2(e)).astype(np.float32)


def _w_pack(W):
    """f32r hi part + fp8e5 DoubleRow pair [(Wr/ASC), (Wl*BSC)] for a weight."""
    W = np.asarray(W, dtype=np.float32)
    Wr = _r12(W)
    Wl = (W.astype(np.float64) - Wr).astype(np.float32)
    pair = np.stack([(Wr / ASC).astype(E5), (Wl * BSC).astype(E5)], axis=1)
    return np.ascontiguousarray(Wr), np.ascontiguousarray(pair)


def kernel(**inputs) -> np.ndarray:
    x = np.ascontiguousarray(np.asarray(inputs["x"], dtype=np.float32))
    Wq = np.asarray(inputs["Wq"], dtype=np.float32)
    Wk = np.asarray(inputs["Wk"], dtype=np.float32)
    Wv = np.asarray(inputs["Wv"], dtype=np.float32)
    bq = np.asarray(inputs["bq"], dtype=np.float32)
    bk = np.asarray(inputs["bk"], dtype=np.float32)
    bv = np.asarray(inputs["bv"], dtype=np.float32)
    pos_bias = np.asarray(inputs["pos_bias"], dtype=np.float32)

    with_bias = bool(np.any(bq) or np.any(bk) or np.any(bv))

    # Diagonal is excluded by the reference (set to -1e9 before top-k); any
    # value below every real score gives the identical top-16 and weights.
    pb_adj = np.ascontiguousarray(pos_bias.copy())
    np.fill_diagonal(pb_adj, DIAGVAL)

    if with_bias not in _PROG_CACHE:
        _PROG_CACHE[with_bias] = build_program(with_bias)
    nc = _PROG_CACHE[with_bias]

    Wqr, Wq8 = _w_pack(Wq)
    Wkr, Wk8 = _w_pack(Wk)
    Wvr = np.ascontiguousarray(_r12(Wv))

    in_maps = []
    for b in range(B):
        xTb = np.ascontiguousarray(x[b, 1:, :].T)
        xr = _r12(xTb)
        xl = (xTb.astype(np.float64) - xr).astype(np.float32)
        x8 = np.ascontiguousarray(
            np.stack([(xl * ASC).astype(E5), (xTb / BSC).astype(E5)], axis=1))
        m = {
            "xr": np.ascontiguousarray(xr), "x8": x8,
            "wqr": Wqr, "wq8": Wq8, "wkr": Wkr, "wk8": Wk8, "wvr": Wvr,
            "pb": pb_adj,
        }
        if with_bias:
            m["bqkv"] = np.ascontiguousarray(np.stack([bq, bk, bv])[None])
        in_maps.append(m)

    res = run_bass_kernel_spmd(nc, in_maps, core_ids=list(range(B)))
    return np.stack([res.results[b]["out"] for b in range(B)]).astype(np.float32)


# revision 23
# speedup vs baseline: 1.0755x; 1.0253x over previous
"""Trainium2 Bass kernel for nn_DavidBeansV2 (sparse wormhole attention).

Math (per batch item b, derived from the reference):
  xp = x[b, 1:, :]                                  # [P, D]
  q  = l2norm(xp @ Wq + bq); k = l2norm(xp @ Wk + bk)
  S  = q @ k.T + pos_bias    (diag forced very negative)
  topk16 per row of S/TEMP -> softmax weights w (zero elsewhere)
  v  = xp @ Wv + bv
  out[b] = (w / rowsum(w)) @ v                      # [P, D]
The multihead gather+combine with routes shared across heads is exactly a
row-sparse [P,P] x [P,D] matmul, so we compute it densely on the PE with a
masked-softmax weight matrix.

Sharding: data-parallel over batch B=8 across the 8 NeuronCores.

Precision: top-16 selection needs ~1e-6-accurate scores (the 16/17
boundary gaps concentrate near zero), which rules out any single-pass
matmul.  Each exact matmul A@B runs as
    r12(A) @ r12(B)            one fp32r pass   (1 cyc/row, 12-bit operands)
  + [Al' | A'] @ [B' | Bl']    one fp8e5 DoubleRow pass (0.5 cyc/row)
where Al = A - r12(A) and the DoubleRow pair computes Al@B + A@Bl with
power-of-two scale splits so both fp8 products land at natural scale and
accumulate into the same PSUM group.  Normalization is factored out of q/k
and applied to the scores (S = (qraw.kraw) * rq[p] * rk[col] + pb) in fp32
vector ops.  V projection is a single fp32r pass; the combine runs in f16.

Schedule (v2): the PE engine is the bottleneck (~136us of work), so every
other engine is kept off PE's critical path:
  - long PE warmup keeps the pstate clock ramped through the DMA-bound start
  - rk broadcast via Pool partition_broadcast (not PE matmuls)
  - pos_bias fully prefetched + pre-scaled by |q| on Act during V-proj
  - per-block: DVE does s*rk and top-16 only; Pool does +pbq and t=s-sz2;
    Act does exp/wT/out; the PE tail (transpose/combine) is interleaved
    into the next block's score matmuls so Act turnaround never stalls PE.
"""

import numpy as np
import ml_dtypes

import concourse.mybir as mybir
import concourse.tile as tile
from concourse import bass_isa
from concourse import bacc
from concourse.bass_utils import run_bass_kernel_spmd
from concourse.masks import make_identity

F32 = mybir.dt.float32
F32R = mybir.dt.float32r
F16 = mybir.dt.float16
F8E5 = mybir.dt.float8e5
AF = mybir.ActivationFunctionType
OP = mybir.AluOpType
DR = mybir.MatmulPerfMode.DoubleRow
E5 = ml_dtypes.float8_e5m2

B, P, D = 8, 1024, 768
TEMP = 0.1
KC = D // 128     # 6 contraction chunks
PB = P // 128     # 8 row blocks
MINVAL = -50.0    # match_replace fill; below any real score, above diag fill
DIAGVAL = -10000.0
ASC = 2.0 ** 6    # xl cross scale: (xl*ASC) @ (Wr/ASC)
BSC = 2.0 ** 10   # Wl cross scale: (x/BSC) @ (Wl*BSC)
NWARM = 22        # PE warmup matmuls ([1,512] each) covering the input DMA


def build_program(with_bias: bool):
    nc = bacc.Bacc(
        "TRN2",
        target_bir_lowering=False,
        debug=False,
        enable_asserts=False,
        num_devices=B,
    )
    xr_d = nc.dram_tensor("xr", [D, P], F32R, kind="ExternalInput").ap()
    x8_d = nc.dram_tensor("x8", [D, 2, P], F8E5, kind="ExternalInput").ap()
    wqr_d = nc.dram_tensor("wqr", [D, D], F32R, kind="ExternalInput").ap()
    wq8_d = nc.dram_tensor("wq8", [D, 2, D], F8E5, kind="ExternalInput").ap()
    wkr_d = nc.dram_tensor("wkr", [D, D], F32R, kind="ExternalInput").ap()
    wk8_d = nc.dram_tensor("wk8", [D, 2, D], F8E5, kind="ExternalInput").ap()
    wvr_d = nc.dram_tensor("wvr", [D, D], F32R, kind="ExternalInput").ap()
    pb = nc.dram_tensor("pb", [P, P], F32, kind="ExternalInput").ap()
    if with_bias:
        bqkv = nc.dram_tensor("bqkv", [1, 3, D], F32, kind="ExternalInput").ap()
    out = nc.dram_tensor("out", [P, D], F16, kind="ExternalOutput").ap()

    with tile.TileContext(nc) as tc:
        consts = tc.alloc_tile_pool(name="consts", bufs=1)
        persist = tc.alloc_tile_pool(name="persist", bufs=1)
        wq_pool = tc.alloc_tile_pool(name="wq_pool", bufs=1)
        inp_pool = tc.alloc_tile_pool(name="inp", bufs=1)
        work2 = tc.alloc_tile_pool(name="work2", bufs=1)
        wk_pool = tc.alloc_tile_pool(name="wk_pool", bufs=1)
        psum2 = tc.alloc_tile_pool(name="psum2", bufs=1, space="PSUM")

        # ---- PE warmup: wide matmuls on a zeroed operand keep the PE busy
        # (and its pstate clock ramping) through the initial input-DMA wait.
        warm_op = consts.tile([128, 448], F16, tag="warm_op")
        nc.gpsimd.memset(warm_op, 0.0)
        warm_ps = psum2.tile([128, 448], F32, tag="warm_ps")
        for _ in range(NWARM):
            nc.tensor.matmul(warm_ps[0:1, :], warm_op[:, 0:1], warm_op,
                             start=True, stop=True)

        ident = consts.tile([128, 128], F16, tag="ident")
        make_identity(nc, ident)
        ones_row = consts.tile([1, 512 if with_bias else 128], F32,
                               tag="ones_row")
        nc.vector.memset(ones_row, 1.0)

        # ---- load inputs, chunked so the first matmuls start early ----
        xr_sb = inp_pool.tile([128, KC, P], F32R, tag="xr_sb", name="xr_sb")
        x8_sb = inp_pool.tile([128, KC, 2, P], F8E5, tag="x8_sb", name="x8_sb")
        wqr_sb = wq_pool.tile([128, KC, D], F32R, tag="wqr_sb", name="wqr_sb")
        wq8_sb = wq_pool.tile([128, KC, 2, D], F8E5, tag="wq8_sb", name="wq8_sb")
        wkr_sb = wk_pool.tile([128, KC, D], F32R, tag="wkr_sb", name="wkr_sb")
        wk8_sb = wk_pool.tile([128, KC, 2, D], F8E5, tag="wk8_sb", name="wk8_sb")

        xr_src = xr_d.rearrange("(o p) f -> p o f", p=128)
        x8_src = x8_d.rearrange("(o p) t f -> p o t f", p=128)
        wqr_src = wqr_d.rearrange("(o p) f -> p o f", p=128)
        wq8_src = wq8_d.rearrange("(o p) t f -> p o t f", p=128)
        wkr_src = wkr_d.rearrange("(o p) f -> p o f", p=128)
        wk8_src = wk8_d.rearrange("(o p) t f -> p o t f", p=128)

        # phase order is k-proj, q-proj, V, blocks.  The first phase streams
        # per-dc chunks (so matmuls start early); later phases use few big
        # DMAs — the SP sequencer spends 650ns dispatching each DMACopy, so
        # many small loads would gate the stream on dispatch, not bandwidth.
        h0, h1 = slice(0, 512), slice(512, P)
        for dc in range(KC):
            nc.sync.dma_start(xr_sb[:, dc, h0], xr_src[:, dc, h0])
            nc.sync.dma_start(wkr_sb[:, dc, :], wkr_src[:, dc, :])
        for t in range(2):
            nc.sync.dma_start(x8_sb[:, :, t, h0], x8_src[:, :, t, h0])
            nc.sync.dma_start(wk8_sb[:, :, t, :], wk8_src[:, :, t, :])
        nc.sync.dma_start(xr_sb[:, 0:3, h1], xr_src[:, 0:3, h1])
        nc.sync.dma_start(xr_sb[:, 3:KC, h1], xr_src[:, 3:KC, h1])
        for t in range(2):
            nc.sync.dma_start(x8_sb[:, :, t, h1], x8_src[:, :, t, h1])
        nc.sync.dma_start(wqr_sb[:, 0:3, :], wqr_src[:, 0:3, :])
        nc.sync.dma_start(wqr_sb[:, 3:KC, :], wqr_src[:, 3:KC, :])
        for t in range(2):
            nc.sync.dma_start(wq8_sb[:, :, t, :], wq8_src[:, :, t, :])
        if with_bias:
            bias_sb = consts.tile([1, 3, D], F32, tag="bias_sb")
            nc.sync.dma_start(bias_sb, bqkv)

        # persistent q/k operands for the scores stage
        q_r = persist.tile([128, KC, P], F32R, tag="q_r", name="q_r")
        k_r = persist.tile([128, KC, P], F32R, tag="k_r", name="k_r")
        q_8 = persist.tile([128, KC, 2, P], F8E5, tag="q_8", name="q_8")
        k_8 = persist.tile([128, KC, 2, P], F8E5, tag="k_8", name="k_8")
        v_sb = persist.tile([128, PB, D], F16, tag="v_sb")
        rk_bcast = persist.tile([128, P], F32, tag="rk_bcast")
        wpack = {"q": (wqr_sb, wq8_sb), "k": (wkr_sb, wk8_sb)}
        rpack = {"q": (q_r, q_8, 0, 1), "k": (k_r, k_8, 1, 0)}
        bidx = {"q": 0, "k": 1}
        rinv_rows = {}

        sq_accs = {}
        mmh_ctr = [0]  # cycles 7 PSUM banks across all projection groups

        def mmh_tile():
            t = psum2.tile([128, 512], F32, tag=f"mmh{mmh_ctr[0] % 7}",
                           name=f"mmh{mmh_ctr[0] % 7}", bufs=1)
            mmh_ctr[0] += 1
            return t

        def emit_proj(nm):
            """Raw projection (f32r + fp8 DoubleRow crosses) + squares.

            The six 128-row output blocks are processed as two ping-pong SETS
            of three PSUM groups: while set B's matmuls run (~5.8us), set A's
            consumers (Act/DVE/Pool splits+squares) drain, so no sweep ever
            stalls on its own consumer chain."""
            ti = bidx[nm]
            wr, w8 = wpack[nm]
            t_r, t_8, l_slot, full_slot = rpack[nm]
            sq_acc = work2.tile([128, P], F32, tag=f"sq_{nm}")
            sq_accs[nm] = sq_acc
            for sl in range(2):
                s = slice(sl * 512, (sl + 1) * 512)
                for half in range(2):
                    dbs = list(range(3 * half, 3 * half + 3))
                    mm = {db: mmh_tile() for db in dbs}
                    # f32r sweep dc-major: consumes each arriving x/w chunk
                    for dc in range(KC):
                        for db in dbs:
                            nc.tensor.matmul(
                                mm[db],
                                wr[:, dc, db * 128:(db + 1) * 128],
                                xr_sb[:, dc, s],
                                start=(dc == 0),
                                stop=False,
                            )
                    # DR sweep dblk-major: groups close staggered
                    for db in dbs:
                        for dc in range(KC):
                            nc.tensor.matmul(
                                mm[db],
                                w8[:, dc, :, db * 128:(db + 1) * 128],
                                x8_sb[:, dc, :, s],
                                start=False,
                                stop=(dc == KC - 1) and not with_bias,
                                perf_mode=DR,
                            )
                        if with_bias:
                            nc.tensor.matmul(
                                mm[db],
                                bias_sb[:, ti, db * 128:(db + 1) * 128],
                                ones_row,
                                start=False,
                                stop=True,
                            )
                        # split raw projection into f32r hi + fp8 pair; the
                        # sq-mul runs FIRST on DVE (it feeds the norm chain,
                        # which gates rk/rq and ultimately the block phase)
                        nc.scalar.activation(t_r[:, db, s], mm[db], AF.Identity)
                        # norm^2 via q_raw*r12(q_raw): 1.3e-6 rel, row-uniform
                        if db == 0:
                            nc.vector.tensor_mul(sq_acc[:, s], mm[db],
                                                 t_r[:, db, s].bitcast(F32))
                        else:
                            sq_full = work2.tile([128, P], F32, tag="allr",
                                                 name="sq_full")
                            nc.vector.tensor_mul(sq_full[:, 0:512], mm[db],
                                                 t_r[:, db, s].bitcast(F32))
                            nc.gpsimd.tensor_add(sq_acc[:, s], sq_acc[:, s],
                                                 sq_full[:, 0:512])
                        nc.scalar.activation(t_8[:, db, full_slot, s], mm[db],
                                             AF.Identity)
                        nc.vector.tensor_sub(t_8[:, db, l_slot, s], mm[db],
                                             t_r[:, db, s].bitcast(F32))

        nq_rows = {}
        # ln and the Newton scratch share one row (disjoint lifetimes); all
        # rows stay base-0 (vector ops need equal input base partitions)
        rowsA = work2.tile([1, P], F32, tag="rowsA", name="rowsA")
        rinv_k_sb = work2.tile([1, P], F32, tag="rinv_k", name="rinv_k")
        rinv_q_sb = work2.tile([1, P], F32, tag="rinv_q", name="rinv_q")
        nq_sb = None  # allocated lazily in q's Newton tail (reuses rinv_k arena)

        def emit_norm_reduce(nm):
            """Partition-reduce the squares on Pool (early, off DVE).
            q's reduce target reuses sq_k's arena (dead after k's reduce)."""
            sq_acc = sq_accs[nm]
            allr = work2.tile([128, P], F32,
                              tag="allr_k" if nm == "k" else "sq_k")
            nc.gpsimd.partition_all_reduce(allr, sq_acc, channels=128,
                                           reduce_op=bass_isa.ReduceOp.add)
            return allr[0:1, :]

        def emit_norm_seed(nm, norm2_sb):
            """Seed 1/|row| = sqrt(1/norm2).  (sqrt's table set also holds
            identity, so projection copies never force an act reload.)"""
            rinv_row = rinv_k_sb if nm == "k" else rinv_q_sb
            nc.vector.reciprocal(rinv_row, norm2_sb)
            nc.scalar.activation(rinv_row, rinv_row, AF.Sqrt)
            rinv_rows[nm] = rinv_row

        def emit_norm_tail(nm, norm2_sb):
            """Two Newton steps (3 fused ops each: spline seeds are too loose
            for the flip-sensitive top-16); for q also |row| itself."""
            rinv_row = rinv_rows[nm]
            rr = rowsA[0:1, :]
            for _ in range(2):
                nc.vector.tensor_mul(rr, rinv_row, rinv_row)
                # rr = (-0.5 * rinv^2) * norm2
                nc.vector.scalar_tensor_tensor(rr, rr, -0.5, norm2_sb,
                                               op0=OP.mult, op1=OP.mult)
                # rinv = (rr + 1.5) * rinv
                nc.vector.scalar_tensor_tensor(rinv_row, rr, 1.5, rinv_row,
                                               op0=OP.add, op1=OP.mult)
            if nm == "q":
                # |q_p| itself: scales pb so the score rows can stay raw
                # (arena of rinv_k, which is dead after k's broadcast)
                nq_row = work2.tile([1, P], F32, tag="rinv_k", name="nq_q")
                nc.vector.tensor_mul(nq_row, norm2_sb, rinv_row)
                nq_rows[nm] = nq_row

        # ---- k projection (chases the x/wk input stream) ----
        emit_proj("k")
        norm2_k = emit_norm_reduce("k")

        # Wv loads into the arena wk_pool frees (queued after the wq loads);
        # pos_bias prefetch follows, landing well before the block phase.
        wk_pool.release()
        wv_pool = tc.alloc_tile_pool(name="wv_pool", bufs=1)
        wvr_sb = wv_pool.tile([128, KC, D], F32R, tag="wvr_sb", name="wvr_sb")
        wvr_src = wvr_d.rearrange("(o p) f -> p o f", p=128)
        nc.sync.dma_start(wvr_sb, wvr_src)

        # ---- q projection (wq loaded during k) ----
        emit_proj("q")
        norm2_q = emit_norm_reduce("q")
        # k's rsqrt seed + Newton run on DVE after ALL q-proj consumers (so
        # they never block the in-order DVE queue ahead of PSUM recycling)
        emit_norm_seed("k", norm2_k)
        emit_norm_tail("k", norm2_k)
        nc.gpsimd.partition_broadcast(rk_bcast, rinv_rows["k"], channels=128)

        # ---- v projection (single f32r pass, natural [p, d] layout) ----
        for pblk in range(PB):
            vh = [mmh_tile() for i in range(2)]
            for dc in range(KC):
                for sl, s, n in ((0, slice(0, 512), 512), (1, slice(512, D), 256)):
                    nc.tensor.matmul(
                        vh[sl][:, :n],
                        xr_sb[:, dc, pblk * 128:(pblk + 1) * 128],
                        wvr_sb[:, dc, s],
                        start=(dc == 0),
                        stop=(dc == KC - 1) and not with_bias,
                    )
            if with_bias:
                for sl, s, n in ((0, slice(0, 512), 512), (1, slice(512, D), 256)):
                    nc.tensor.matmul(
                        vh[sl][:, :n],
                        ones_row[:, :128],
                        bias_sb[:, 2, s],
                        start=False,
                        stop=True,
                    )
            nc.scalar.activation(v_sb[:, pblk, 0:512], vh[0], AF.Identity)
            nc.scalar.activation(v_sb[:, pblk, 512:D], vh[1][:, :256], AF.Identity)
            if pblk == 1:
                # q's seed + Newton chain runs on the otherwise-idle DVE here
                emit_norm_seed("q", norm2_q)
                emit_norm_tail("q", norm2_q)
        wv_pool.release()

        # ---- rq relayout: [rq | nq] rows -> per-partition columns ----
        # (kept on PE: 16 one-column matmuls; q's Newton finished during V)
        rqx_cols = persist.tile([128, 2, PB], F32, tag="rqx_cols")
        rqT_cols = persist.tile([128, PB], F32, tag="rqT_cols")
        nrqT_cols = persist.tile([128, PB], F32, tag="nrqT_cols")
        rq_ps = psum2.tile([128, 448], F32, tag="warm_ps", name="rq_ps",
                           bufs=1)
        for si, row in ((0, rinv_rows["q"]), (1, nq_rows["q"])):
            for j in range(PB):
                nc.tensor.matmul(
                    rq_ps[:, si * PB + j:si * PB + j + 1],
                    row[:, j * 128:(j + 1) * 128],
                    ones_row[:, 0:1],
                    start=True,
                    stop=True,
                )
        nc.scalar.activation(
            rqx_cols, rq_ps[:, 0:2 * PB].rearrange("p (a b) -> p a b", a=2),
            AF.Identity)
        # rq/TEMP and -rq/TEMP columns for the exp scale/bias
        nc.vector.tensor_scalar_mul(rqT_cols, rqx_cols[:, 0, :], 1.0 / TEMP)
        nc.vector.tensor_scalar_mul(nrqT_cols, rqx_cols[:, 0, :], -1.0 / TEMP)

        work2.release()
        inp_pool.release()
        wq_pool.release()
        psum2.release()

        # ---- per row-block: scores, top-16 softmax, combine ----
        work3 = tc.alloc_tile_pool(name="work3", bufs=2)
        wpool = tc.alloc_tile_pool(name="wpool", bufs=3)
        pb_pool = tc.alloc_tile_pool(name="pb_pool", bufs=3)
        psum3 = tc.alloc_tile_pool(name="psum3", bufs=1, space="PSUM")

        state = {}

        def emit_pb_load(pblk):
            """Prefetch one pos_bias row block (3-deep rotation)."""
            pb_t = pb_pool.tile([128, P], F32, tag="pb_sb")
            nc.sync.dma_start(pb_t, pb[pblk * 128:(pblk + 1) * 128, :])
            state.setdefault(pblk, {})["pb"] = pb_t

        def emit_prep_half(pblk, sl):
            """S matmuls for one 512-half + DVE rk-scale + Pool pb-add."""
            pbs = slice(pblk * 128, (pblk + 1) * 128)
            if sl == 0:
                s_sb = work3.tile([128, P], F32, tag="s_sb")
                state.setdefault(pblk, {}).update(s_sb=s_sb)
            else:
                s_sb = state[pblk]["s_sb"]
            s_ps = psum3.tile([128, 512], F32, tag="s_psh", name="s_psh",
                              bufs=4)
            s = slice(sl * 512, (sl + 1) * 512)
            for dc in range(KC):
                nc.tensor.matmul(
                    s_ps,
                    q_r[:, dc, pbs],
                    k_r[:, dc, s],
                    start=(dc == 0),
                    stop=False,
                )
            for dc in range(KC):
                nc.tensor.matmul(
                    s_ps,
                    q_8[:, dc, :, pbs],
                    k_8[:, dc, :, s],
                    start=False,
                    stop=(dc == KC - 1),
                    perf_mode=DR,
                )
            # s' = Sraw*rk + pb*|q| ; rows stay scaled by |q_p|.  The |q|
            # scale folds into the Pool add (one scalar_tensor_tensor).
            nc.vector.tensor_mul(s_sb[:, s], s_ps, rk_bcast[:, s])
            nc.gpsimd.scalar_tensor_tensor(
                s_sb[:, s], state[pblk]["pb"][:, s],
                rqx_cols[:, 1, pblk:pblk + 1], s_sb[:, s],
                op0=OP.mult, op1=OP.add)

        def emit_topk(pblk):
            """top-16 (DVE) + t = s - sz2 (Pool) + exp (Act) for pblk."""
            s_sb = state[pblk]["s_sb"]
            # top-16 per row: two rounds of max8 + match_replace
            m8a = work3.tile([128, 8], F32, tag="m8a")
            nc.vector.max(m8a, s_sb)
            sz1 = work3.tile([128, P], F32, tag="sz1")
            nc.vector.match_replace(sz1, in_to_replace=m8a, in_values=s_sb,
                                    imm_value=MINVAL)
            m8b = work3.tile([128, 8], F32, tag="m8b")
            nc.vector.max(m8b, sz1)
            sz2 = work3.tile([128, P], F32, tag="sz2")
            nc.vector.match_replace(sz2, in_to_replace=m8b, in_values=sz1,
                                    imm_value=MINVAL)
            # w = exp((T + MINVAL - m) * rq/TEMP); off-top entries underflow
            ebias = work3.tile([128, 1], F32, tag="ebias")
            nc.vector.tensor_scalar_add(ebias, m8a[:, 0:1], -MINVAL)
            nc.vector.tensor_mul(ebias, ebias, nrqT_cols[:, pblk:pblk + 1])
            if pblk >= 1:
                emit_recip(pblk - 1)
            # T = s - sz2: 0 off the top-16, s - MINVAL on it (Pool)
            t_sb = work3.tile([128, P], F32, tag="t_sb")
            nc.gpsimd.tensor_sub(t_sb, s_sb, sz2)
            w_sb = wpool.tile([128, P], F16, tag="w_sb", bufs=4)
            den = wpool.tile([128, 1], F32, tag="den", bufs=4)
            nc.scalar.activation(w_sb, t_sb, AF.Exp, bias=ebias,
                                 scale=rqT_cols[:, pblk:pblk + 1],
                                 accum_out=den)
            state[pblk].update(w_sb=w_sb, den=den)

        def emit_recip(pblk):
            rden = wpool.tile([128, 1], F32, tag="rden", bufs=4)
            nc.vector.reciprocal(rden, state[pblk]["den"])
            state[pblk]["rden"] = rden

        def emit_tp(pblk):
            """PE transpose of w + Act copy to SBUF."""
            w_sb = state[pblk]["w_sb"]
            tp_ps = psum3.tile([128, P], F16, tag="tp_ps", name="tp_ps",
                               bufs=2)
            for qc in range(PB):
                nc.tensor.transpose(
                    tp_ps[:, qc * 128:(qc + 1) * 128],
                    w_sb[:, qc * 128:(qc + 1) * 128],
                    ident,
                )
            wT_sb = work3.tile([128, P], F16, tag="wT_sb")
            nc.scalar.activation(wT_sb, tp_ps, AF.Identity)
            state[pblk]["wT"] = wT_sb

        def emit_combine(pblk):
            """PE combine with v, scale by 1/den, store."""
            wT_sb, rden = state[pblk]["wT"], state[pblk]["rden"]
            pbs = slice(pblk * 128, (pblk + 1) * 128)
            o_ps = psum3.tile([128, D], F32, tag="o_ps", name="o_ps", bufs=1)
            for qc in range(PB):
                for sl, s in ((0, slice(0, 512)), (1, slice(512, D))):
                    nc.tensor.matmul(
                        o_ps[:, s],
                        wT_sb[:, qc * 128:(qc + 1) * 128],
                        v_sb[:, qc, s],
                        start=(qc == 0),
                        stop=(qc == PB - 1),
                    )
            out_sb = work3.tile([128, D], F16, tag="out_sb")
            nc.scalar.activation(out_sb, o_ps, AF.Identity, scale=rden)
            nc.sync.dma_start(out[pbs, :], out_sb)

        # software pipeline: per macro-step i the PE runs
        #   prep_i(h0) | tp_{i-3} | prep_i(h1) | combine_{i-3}
        # so the Act wT turnaround for block i-3 overlaps prep_i(h1), and
        # DVE's topk_{i-1} overlaps both prep halves.  The tail lag of 3
        # absorbs the pipeline-fill bubble of the first block's DVE chain.
        emit_pb_load(0)
        emit_pb_load(1)
        emit_pb_load(2)
        for i in range(PB):
            emit_prep_half(i, 0)
            if i >= 3:
                emit_tp(i - 3)
            if i >= 1:
                emit_topk(i - 1)
            emit_prep_half(i, 1)
            if i >= 3:
                emit_combine(i - 3)
            if i + 3 <= PB - 1:
                emit_pb_load(i + 3)
        # drain: tails for blocks 5,6,7 and the last top-k chain
        emit_tp(PB - 3)
        emit_topk(PB - 1)
        emit_combine(PB - 3)
        emit_tp(PB - 2)
        emit_recip(PB - 1)
        emit_combine(PB - 2)
        emit_tp(PB - 1)
        emit_combine(PB - 1)

        pb_pool.release()
        wpool.release()
        work3.release()
        psum3.release()
        persist.release()
        consts.release()

    nc.finalize()
    return nc


_PROG_CACHE = {}


def _r12(a):
    """Round fp32 values to 12 explicit mantissa bits (= f32r rounding)."""
    m, e = np.frexp(np.asarray(a, np.float64))
    return ((np.round(m * 4096.0) / 4096.0) * np.exp2(e)).astype(np.float32)


def _w_pack(W):
    """f32r hi part + fp8e5 DoubleRow pair [(Wr/ASC), (Wl*BSC)] for a weight."""
    W = np.asarray(W, dtype=np.float32)
    Wr = _r12(W)
    Wl = (W.astype(np.float64) - Wr).astype(np.float32)
    pair = np.stack([(Wr / ASC).astype(E5), (Wl * BSC).astype(E5)], axis=1)
    return np.ascontiguousarray(Wr), np.ascontiguousarray(pair)


def kernel(**inputs) -> np.ndarray:
    x = np.ascontiguousarray(np.asarray(inputs["x"], dtype=np.float32))
    Wq = np.asarray(inputs["Wq"], dtype=np.float32)
    Wk = np.asarray(inputs["Wk"], dtype=np.float32)
    Wv = np.asarray(inputs["Wv"], dtype=np.float32)
    bq = np.asarray(inputs["bq"], dtype=np.float32)
    bk = np.asarray(inputs["bk"], dtype=np.float32)
    bv = np.asarray(inputs["bv"], dtype=np.float32)
    pos_bias = np.asarray(inputs["pos_bias"], dtype=np.float32)

    with_bias = bool(np.any(bq) or np.any(bk) or np.any(bv))

    # Diagonal is excluded by the reference (set to -1e9 before top-k); any
    # value below every real score gives the identical top-16 and weights.
    pb_adj = np.ascontiguousarray(pos_bias.copy())
    np.fill_diagonal(pb_adj, DIAGVAL)

    if with_bias not in _PROG_CACHE:
        _PROG_CACHE[with_bias] = build_program(with_bias)
    nc = _PROG_CACHE[with_bias]

    Wqr, Wq8 = _w_pack(Wq)
    Wkr, Wk8 = _w_pack(Wk)
    Wvr = np.ascontiguousarray(_r12(Wv))

    in_maps = []
    for b in range(B):
        xTb = np.ascontiguousarray(x[b, 1:, :].T)
        xr = _r12(xTb)
        xl = (xTb.astype(np.float64) - xr).astype(np.float32)
        x8 = np.ascontiguousarray(
            np.stack([(xl * ASC).astype(E5), (xTb / BSC).astype(E5)], axis=1))
        m = {
            "xr": np.ascontiguousarray(xr), "x8": x8,
            "wqr": Wqr, "wq8": Wq8, "wkr": Wkr, "wk8": Wk8, "wvr": Wvr,
            "pb": pb_adj,
        }
        if with_bias:
            m["bqkv"] = np.ascontiguousarray(np.stack([bq, bk, bv])[None])
        in_maps.append(m)

    res = run_bass_kernel_spmd(nc, in_maps, core_ids=list(range(B)))
    return np.stack([res.results[b]["out"] for b in range(B)]).astype(np.float32)
